# revision 20
# baseline (speedup 1.0000x reference)
"""Trainium2 Bass kernel for the ADNet advection-diffusion step.

Input : inputs [16, 4, 1024, 1024] f32  (channels: u, v, T_prev, RaQ_Ra)
Output: (T_new [16, 1, 1024, 1024] f32, dt scalar f32)

Sharding: data-parallel over batch. Core i handles images [2i, 2i+1]
(contiguous slice of the input, zero-copy). The only cross-core
communication is a 4-byte AllReduce(max) for the CFL timestep.

Per-core dataflow (per 128-row block of each 1024x1024 image):
  - y-direction stencils (upwind diffs s1y/s2y of T, and the [1,2,1]
    smoother Sy of Sx) run on the TensorEngine as banded-matrix matmuls
    in bf16, with single-corner matrices accumulating the 1-row halo
    from the neighboring block directly in PSUM.
  - x-direction diffs/smoother + upwind products run on the Vector
    engine (fused scalar_tensor_tensor where possible, bf16).
  - relu masks and PSUM evictions run on the Scalar (ACT) engine.
  - |u|,|v| max-reduction runs on GpSimd; global max via AllReduce.
"""

import threading

import numpy as np
import ml_dtypes

import concourse.bacc as bacc
import concourse.bass as bass
import concourse.mybir as mybir
import concourse.tile as tile
from concourse.bass_utils import run_bass_kernel_spmd

F32 = mybir.dt.float32
BF16 = mybir.dt.bfloat16
AF = mybir.ActivationFunctionType
OP = mybir.AluOpType

N_CORES = 8
B_PER_CORE = 2          # images per core
H = 1024
W = 1024
NB = H // 128           # 8 row-blocks per image
HALF = 512              # matmul moving free-dim limit

DX = np.float32(1.0) / np.float32(126.0)
CN_MAX = np.float32(0.1)
# dt_diffuse exactly as the reference computes it in f32
_D2 = DX * DX
DT_DIFF = np.float32(0.5) * (_D2 * _D2) / (_D2 + _D2)
C_ADV = np.float32(0.5) * CN_MAX * DX          # dt_advect = C_ADV / uv_mag
C_K2 = -np.float32(1.0) / DX                   # k2 = -dt/dx
C_A = np.float32(0.25) / _D2                   # a  = 0.25*dt/dx^2

# ---------------------------------------------------------------------------
# Host-built stationary matrices (lhsT layout: out = M @ x  =>  lhsT = M.T)
# ---------------------------------------------------------------------------

def _build_mats() -> np.ndarray:
    I = np.eye(128, dtype=np.float32)
    sub = np.diag(np.ones(127, dtype=np.float32), -1)   # D[i, i-1] = 1
    sup = np.diag(np.ones(127, dtype=np.float32), 1)    # D[i, i+1] = 1

    mats = []

    # 0: s1y mid  (out[i] = T[i] - T[i-1]; row 0 pairs with up-corner)
    m = I - sub
    mats.append(m)
    # 1: s1y top  (image top: s1y[0] = 0)
    m = I - sub
    m[0, :] = 0.0
    mats.append(m)
    # 2: s1y up-corner (applied to prev block: out[0] -= Tprev[127])
    m = np.zeros((128, 128), dtype=np.float32)
    m[0, 127] = -1.0
    mats.append(m)

    # 3: s2y mid  (out[i] = T[i] - T[i+1]; row 127 pairs with dn-corner)
    m = I - sup
    mats.append(m)
    # 4: s2y bot  (image bottom: s2y[127] = 0)
    m = I - sup
    m[127, :] = 0.0
    mats.append(m)
    # 5: s2y dn-corner (applied to next block: out[127] -= Tnext[0])
    m = np.zeros((128, 128), dtype=np.float32)
    m[127, 0] = -1.0
    mats.append(m)

    # 6: Sy mid  (out[i] = X[i-1] + 2 X[i] + X[i+1])
    m = 2.0 * I + sub + sup
    mats.append(m)
    # 7: Sy top  (row 0: 3 X[0] + X[1])
    m = 2.0 * I + sub + sup
    m[0, 0] = 3.0
    mats.append(m)
    # 8: Sy bot  (row 127: X[126] + 3 X[127])
    m = 2.0 * I + sub + sup
    m[127, 127] = 3.0
    mats.append(m)
    # 9: Sy up-corner (out[0] += Xprev[127])
    m = np.zeros((128, 128), dtype=np.float32)
    m[0, 127] = 1.0
    mats.append(m)
    # 10: Sy dn-corner (out[127] += Xnext[0])
    m = np.zeros((128, 128), dtype=np.float32)
    m[127, 0] = 1.0
    mats.append(m)

    # 11..15: doubled Sy set, applied to Tn (Sy(Sx) = Sy(x3) + 2 Sy(Tn))
    for i in (6, 7, 8, 9, 10):
        mats.append(2.0 * mats[i])

    lhsT = np.stack([m.T for m in mats])          # [16, 128, 128]
    return lhsT.astype(ml_dtypes.bfloat16)

N_MATS = 16

# ---------------------------------------------------------------------------
# Device module builder
# ---------------------------------------------------------------------------

def build_module() -> bacc.Bacc:
    nc = bacc.Bacc(
        "TRN2",
        target_bir_lowering=False,
        debug=False,
        num_devices=N_CORES,
    )

    x = nc.dram_tensor("x", [B_PER_CORE, 4, H, W], BF16, kind="ExternalInput").ap()
    mats = nc.dram_tensor("mats", [N_MATS, 128, 128], BF16, kind="ExternalInput").ap()
    out = nc.dram_tensor("out", [B_PER_CORE, H, W], BF16, kind="ExternalOutput").ap()
    dtout = nc.dram_tensor("dtout", [1, 1], F32, kind="ExternalOutput").ap()

    groups = [list(range(N_CORES))]

    with tile.TileContext(nc) as tc:
        consts = tc.alloc_tile_pool(name="consts", bufs=1)
        uvp = tc.alloc_tile_pool(name="uv", bufs=8)
        tbp = tc.alloc_tile_pool(name="tb", bufs=2 * NB + 1)
        advp = tc.alloc_tile_pool(name="adv", bufs=2 * NB)
        tmp = tc.alloc_tile_pool(name="tmp", bufs=3)
        tnp = tc.alloc_tile_pool(name="tn", bufs=4)
        sxp = tc.alloc_tile_pool(name="sx", bufs=4)
        rp = tc.alloc_tile_pool(name="r", bufs=3)
        op_ = tc.alloc_tile_pool(name="outp", bufs=3)
        psum = tc.alloc_tile_pool(name="ps", bufs=8, space="PSUM")
        tiny = tc.alloc_tile_pool(name="tiny", bufs=4)
        dram = tc.alloc_tile_pool(name="dram", bufs=1, space="DRAM")

        # --- constants -----------------------------------------------------
        mat_t = []
        for k in range(N_MATS):
            mt = consts.tile([128, 128], BF16, tag=f"mat{k}")
            nc.gpsimd.dma_start(out=mt[:], in_=mats[k])
            mat_t.append(mt)

        # per-block |u|,|v| free-dim maxima land in one tile (no serial chain)
        red_all = consts.tile([128, NB * B_PER_CORE], F32, tag="red_all")

        dt_scalars = []

        def _emit_dt_chain():
            macc = tiny.tile([128, 1], F32, tag="macc", name="macc")
            nc.vector.tensor_reduce(macc[:], red_all[:], mybir.AxisListType.X,
                                    OP.max)
            mall = tiny.tile([128, 1], F32, tag="mall", name="mall")
            nc.gpsimd.partition_all_reduce(
                mall[:], macc[:], channels=128,
                reduce_op=bass.bass_isa.ReduceOp.max)
            cc_in = dram.tile([1, 1], F32, tag="cc_in", name="cc_in")
            cc_out = dram.tile([1, 1], F32, tag="cc_out", name="cc_out")
            nc.gpsimd.dma_start(out=cc_in[:], in_=mall[0:1, 0:1])
            nc.gpsimd.collective_compute(
                "AllReduce", OP.max, replica_groups=groups,
                ins=[cc_in.opt()], outs=[cc_out.opt()],
            )
            uvm = tiny.tile([1, 1], F32, tag="uvm", name="uvm")
            nc.gpsimd.dma_start(out=uvm[:], in_=cc_out[:])
            uvmb = tiny.tile([128, 1], F32, tag="uvmb", name="uvmb")
            nc.gpsimd.partition_broadcast(uvmb[:], uvm[:], channels=128)

            inv = tiny.tile([128, 1], F32, tag="inv", name="inv")
            nc.vector.reciprocal(inv[:], uvmb[:])
            dt = tiny.tile([128, 1], F32, tag="dt", name="dt")
            nc.vector.tensor_scalar(dt[:], inv[:], float(C_ADV), float(DT_DIFF),
                                    OP.mult, OP.min)
            k2 = tiny.tile([128, 1], F32, tag="k2", name="k2")
            nc.vector.tensor_scalar(k2[:], dt[:], float(C_K2), None, OP.mult)
            a_c = tiny.tile([128, 1], F32, tag="a_c", name="a_c")
            nc.vector.tensor_scalar(a_c[:], dt[:], float(C_A), None, OP.mult)
            c1 = tiny.tile([128, 1], F32, tag="c1", name="c1")
            nc.vector.tensor_scalar(c1[:], a_c[:], -16.0, 1.0, OP.mult, OP.add)
            nc.sync.dma_start(out=dtout[:], in_=dt[0:1, 0:1])
            dt_scalars.append((dt, k2, a_c, c1))

        # ============  LOOP 0: loads + |uv| reduces + dt chain  ============
        tb_all = [[None] * NB for _ in range(B_PER_CORE)]
        adv_all = [[None] * NB for _ in range(B_PER_CORE)]
        uv_all = [[None] * NB for _ in range(B_PER_CORE)]

        for im in range(B_PER_CORE):
            for k in range(NB):
                r0 = 128 * k
                uv = uvp.tile([128, 2 * W], BF16, tag="uv")
                nc.sync.dma_start(out=uv[:, 0:W], in_=x[im, 0, r0:r0 + 128, :])
                nc.sync.dma_start(out=uv[:, W:2 * W], in_=x[im, 1, r0:r0 + 128, :])
                uv_all[im][k] = uv
                tb = tbp.tile([128, W], BF16, tag="tb")
                nc.gpsimd.dma_start(out=tb[:], in_=x[im, 2, r0:r0 + 128, :])
                tb_all[im][k] = tb
                bi = im * NB + k
                nc.vector.tensor_reduce(
                    red_all[:, bi:bi + 1], uv[:], mybir.AxisListType.X, OP.max,
                    apply_absolute_value=True,
                )
        _emit_dt_chain()

        # =====================  PHASE 1 (dt-independent)  ==================
        for im in range(B_PER_CORE):
            vpn_ring = [None] * NB
            for k in range(NB + 1):
                if k < NB:
                    r0 = 128 * k
                    uv = uv_all[im][k]

                    # upwind masks: upn = [relu(u) | relu(-u)], vpn likewise
                    upn = tmp.tile([128, 2 * W], BF16, tag="upn", bufs=4)
                    vpn = tmp.tile([128, 2 * W], BF16, tag=f"vpn{k % 3}", bufs=1)
                    nc.scalar.activation(upn[:, 0:W], uv[:, 0:W], AF.Relu)
                    nc.scalar.activation(upn[:, W:2 * W], uv[:, 0:W], AF.Relu,
                                         scale=-1.0)
                    nc.scalar.activation(vpn[:, 0:W], uv[:, W:2 * W], AF.Relu)
                    nc.scalar.activation(vpn[:, W:2 * W], uv[:, W:2 * W], AF.Relu,
                                         scale=-1.0)
                    vpn_ring[k] = vpn

                    tb = tb_all[im][k]

                    # x-direction upwind diffs in bf16: s12 = [s1 | s2]
                    s12 = tmp.tile([128, 2 * W], BF16, tag="s12", bufs=4)
                    nc.vector.tensor_tensor(
                        s12[:, 1:W], tb[:, 1:W], tb[:, 0:W - 1], OP.subtract)
                    nc.vector.memset(s12[:, 0:1], 0.0)
                    nc.vector.tensor_tensor(
                        s12[:, W:2 * W - 1], tb[:, 0:W - 1], tb[:, 1:W], OP.subtract)
                    nc.vector.memset(s12[:, 2 * W - 1:2 * W], 0.0)

                    nc.vector.tensor_mul(s12[:], upn[:], s12[:])
                    adv = advp.tile([128, W], BF16, tag="adv")
                    nc.gpsimd.tensor_add(adv[:], s12[:, 0:W], s12[:, W:2 * W])
                    adv_all[im][k] = adv

                if k >= 1:
                    kk = k - 1
                    # PE y-direction upwind diffs for block kk
                    ps = [psum.tile([128, HALF], F32, tag="ps",
                                    name=f"ps1_{im}_{k}_{j}") for j in range(4)]
                    for h in range(2):
                        c0 = HALF * h
                        rhs_c = tb_all[im][kk][:, c0:c0 + HALF]
                        if kk == 0:
                            nc.tensor.matmul(ps[h][:], mat_t[1][:], rhs_c,
                                             start=True, stop=True)
                        else:
                            nc.tensor.matmul(ps[h][:], mat_t[0][:], rhs_c,
                                             start=True, stop=False)
                            nc.tensor.matmul(
                                ps[h][:], mat_t[2][:],
                                tb_all[im][kk - 1][:, c0:c0 + HALF],
                                start=False, stop=True)
                        if kk == NB - 1:
                            nc.tensor.matmul(ps[2 + h][:], mat_t[4][:], rhs_c,
                                             start=True, stop=True)
                        else:
                            nc.tensor.matmul(ps[2 + h][:], mat_t[3][:], rhs_c,
                                             start=True, stop=False)
                            nc.tensor.matmul(
                                ps[2 + h][:], mat_t[5][:],
                                tb_all[im][kk + 1][:, c0:c0 + HALF],
                                start=False, stop=True)

                    # evict both y-diffs into one [128, 2W] bf16 tile
                    s12y = tmp.tile([128, 2 * W], BF16, tag="s12y", bufs=3)
                    for h in range(2):
                        c0 = HALF * h
                        nc.scalar.activation(s12y[:, c0:c0 + HALF], ps[h][:], AF.Copy)
                        nc.scalar.activation(s12y[:, W + c0:W + c0 + HALF],
                                             ps[2 + h][:], AF.Copy)

                    nc.vector.tensor_mul(s12y[:], vpn_ring[kk][:], s12y[:])
                    adv = adv_all[im][kk]
                    nc.gpsimd.tensor_add(adv[:], adv[:], s12y[:, 0:W])
                    nc.gpsimd.tensor_add(adv[:], adv[:], s12y[:, W:2 * W])

        dt, k2, a_c, c1 = dt_scalars[0]

        # =====================  PHASE 2  ===================================
        for im in range(B_PER_CORE):
            tn_ring = [None] * NB
            sx_ring = [None] * NB
            for k in range(NB + 1):
                if k < NB:
                    adv = adv_all[im][k]
                    tb = tb_all[im][k]
                    nc.vector.tensor_scalar(adv[:], adv[:], k2[:, 0:1], None, OP.mult)
                    tn = tnp.tile([128, W], BF16, tag="tn")
                    nc.vector.tensor_add(tn[:], adv[:], tb[:])
                    tn_ring[k] = tn

                    x3 = sxp.tile([128, W], BF16, tag="x3")
                    nc.vector.tensor_tensor(
                        x3[:, 1:W - 1], tn[:, 0:W - 2], tn[:, 2:W], OP.add)
                    nc.vector.tensor_add(x3[:, 0:1], tn[:, 0:1], tn[:, 1:2])
                    nc.vector.tensor_add(x3[:, W - 1:W], tn[:, W - 2:W - 1],
                                         tn[:, W - 1:W])
                    sx_ring[k] = x3

                if k >= 1:
                    kk = k - 1
                    r0 = 128 * kk
                    ps = [psum.tile([128, HALF], F32, tag="ps",
                                    name=f"ps2_{im}_{k}_{j}") for j in range(2)]
                    for h in range(2):
                        c0 = HALF * h
                        mms = []
                        for ring, base in ((sx_ring, 6), (tn_ring, 11)):
                            cm = base if 0 < kk < NB - 1 else (
                                base + 1 if kk == 0 else base + 2)
                            mms.append((cm, ring[kk]))
                            if kk > 0:
                                mms.append((base + 3, ring[kk - 1]))
                            if kk < NB - 1:
                                mms.append((base + 4, ring[kk + 1]))
                        for j, (mi, rtile) in enumerate(mms):
                            nc.tensor.matmul(ps[h][:], mat_t[mi][:],
                                             rtile[:, c0:c0 + HALF],
                                             start=(j == 0),
                                             stop=(j == len(mms) - 1))

                    # evict Sy with the a_c scale folded in
                    syb = tmp.tile([128, W], BF16, tag="syb")
                    for h in range(2):
                        c0 = HALF * h
                        nc.scalar.activation(syb[:, c0:c0 + HALF], ps[h][:],
                                             AF.Copy, scale=a_c[:, 0:1])

                    rt = rp.tile([128, W], BF16, tag="r")
                    nc.sync.dma_start(out=rt[:], in_=x[im, 3, r0:r0 + 128, :])
                    rpb = tmp.tile([128, W], BF16, tag="rpb")
                    nc.vector.tensor_scalar(rpb[:], rt[:], dt[:, 0:1], None, OP.mult)
                    tc1 = tmp.tile([128, W], BF16, tag="tc1")
                    nc.vector.tensor_scalar(tc1[:], tn_ring[kk][:], c1[:, 0:1],
                                            None, OP.mult)
                    nc.vector.tensor_add(tc1[:], tc1[:], rpb[:])
                    ot = op_.tile([128, W], BF16, tag="ot")
                    nc.vector.tensor_add(ot[:], syb[:], tc1[:])
                    nc.sync.dma_start(out=out[im, r0:r0 + 128, :], in_=ot[:])

        for _pool in reversed((consts, uvp, tbp, advp, tmp, tnp, sxp,
                               rp, op_, psum, tiny, dram)):
            _pool.release()

    nc.compile()
    return nc


# ---------------------------------------------------------------------------
# Host wrapper
# ---------------------------------------------------------------------------

_cache_lock = threading.Lock()
_cached = {}


def _get_module():
    with _cache_lock:
        if "nc" not in _cached:
            _cached["nc"] = build_module()
            _cached["mats"] = _build_mats()
    return _cached["nc"], _cached["mats"]


def _run(inputs: np.ndarray, **spmd_kwargs):
    assert inputs.shape == (16, 4, H, W), inputs.shape
    nc, mats = _get_module()
    x_bf = np.asarray(inputs, dtype=np.float32).astype(ml_dtypes.bfloat16)
    in_maps = [
        {"x": x_bf[B_PER_CORE * i:B_PER_CORE * (i + 1)], "mats": mats}
        for i in range(N_CORES)
    ]
    res = run_bass_kernel_spmd(
        nc, in_maps, core_ids=list(range(N_CORES)), **spmd_kwargs
    )
    outs = res.results
    t_new = np.empty((16, 1, H, W), dtype=np.float32)
    for i in range(N_CORES):
        t_new[B_PER_CORE * i:B_PER_CORE * (i + 1), 0] = (
            outs[i]["out"].astype(np.float32))
    dt = np.float32(outs[0]["dtout"][0, 0])
    return (t_new, dt), res


def kernel(inputs: np.ndarray):
    (t_new, dt), _ = _run(inputs)
    return t_new, dt
